# revision 4
# baseline (speedup 1.0000x reference)
"""Trainium2 Bass kernel v2 for nn_MeshLoss2D (chamfer min-distance mesh loss).

Computation: refine a (B,3,32,32) mesh grid by bilinear factor 3 to (B,3,94,94),
then for every point-cloud point (B,3,4096) find min squared distance to any
refined mesh point, and return the mean over all B*4096 points.

Sharding: 8 cores = (batch b, pc half h); each core handles 2048 pc points of
one batch and that batch's full mesh (8836 points, exact - no padding).

v2 design (116129 ns baseline -> 105719 ns modeled):
  - 5-row fp16 "perturbed-exact" matmuls: quantize mesh and pc points to
    fp16 (m~, p~) and compute min_j d(p~, m~_j) EXACTLY: rows
    a=[-2p~(3), 1, 1], b=[m~(3), s_hi, s_lo] with s = ||m~||^2 computed in
    fp32 and split hi/lo fp16. fp16 x fp16 products accumulate exactly in
    PSUM fp32, so the only error is the ~1e-4 geometric perturbation
    (measured end-to-end rel err ~4e-4). Host adds back ||p~||^2. This
    replaces the old 12-row hi/lo split (fewer rows, simpler staging).
  - per pc-tile: 9 PSUM windows ([1024]*7 + [868, 800], exact 8836 cols, no
    padding) from a 4-deep [128,1024] PSUM ring, each routed through one of
    two paths by a tuned global pattern (8A:5D - ACT and DVE are the only
    engines that can read PSUM on real TRN2; GPSIMD tensor ops fail walrus
    codegen and DMA accum is add-only):
      D: DVE tensor_reduce(min) straight from PSUM       (~1.16 ns/col)
      A: ACT copy PSUM->SBUF fp16 into a contiguous      (~1.01 ns/col ACT)
         per-tile ebuf; ONE merged 4x-packed DVE
         tensor_scalar(min, accum min) per tile          (~0.27 ns/col DVE)
  - startup (was 21.7us, now ~12): single packed fp16 input DMA (grid+interp
    matrix); refine runs fp16 on the PE; staging reads refine PSUM directly
    (ACT copy to fp16 + DVE squares); SBUF->SBUF flatten as per-(row-slab,
    rhs-row) DMAs spread across SP/ACT/GPSIMD queues so the first windows
    start while later slabs are still in flight; split result writeback.
"""

import os
import sys

for _p in ("/opt/trn_rl_repo", "/opt/trn_rl_repo/concourse"):
    if _p not in sys.path:
        sys.path.insert(0, _p)

import numpy as np

B, C, H, W = 4, 3, 32, 32
FACTOR = 3
OH = (H - 1) * FACTOR + 1        # 94
N_MESH = OH * OH                 # 8836
M_TOTAL = 4096
N_CORES = 8
M_CORE = M_TOTAL * B // N_CORES  # 2048 pc points per core
PC_TILES = M_CORE // 128         # 16

# window widths per pc-tile: 7*1024 + 868 + 800 = 8836; every matmul chunk
# starts on a PSUM bank boundary (matmuls may not cross banks)
WIN_WIDTHS = [1024] * 7 + [868, 800]
# global rotating window pattern, tuned 8A:5D (see module docstring)
PATTERN = "DAADADAADADAA"
# flatten row slabs (rows of the 94x94 refined grid per DMA)
SLABS = [(0, 12), (12, 40), (40, 94)]
N_WARM = 0                       # dummy PE matmuls (tested: do not help)

_BUILT = {}
LAST_RESULTS = None


def _interp_matrix():
    """R [OH, H] fp32 with R[o, y0]=1-w, R[o, y0+1]=w replicating reference
    fp32 arithmetic (ys = arange(oh)/3 in fp32)."""
    ys = np.arange(OH, dtype=np.float32) / np.float32(FACTOR)
    y0 = np.clip(np.floor(ys).astype(np.int64), 0, H - 2)
    wy = ys - y0.astype(np.float32)
    R = np.zeros((OH, H), dtype=np.float32)
    R[np.arange(OH), y0] = np.float32(1.0) - wy
    R[np.arange(OH), y0 + 1] += wy
    return R


def _chunks(width):
    """Matmul chunk widths: each starts on a 512-col PSUM bank boundary and
    is >= 256 wide so f32r runs at full rate."""
    assert 768 < width <= 1024
    return [512, width - 512]


def _build_kernel(pattern=PATTERN, win_widths=None, psum_bufs=4, evac_bufs=6,
                  n_warm=N_WARM):
    from concourse import bacc, mybir
    import concourse.tile as tile

    if win_widths is None:
        win_widths = WIN_WIDTHS
    assert sum(win_widths) == N_MESH

    f32 = mybir.dt.float32
    f32r = mybir.dt.float32r
    f16 = mybir.dt.float16
    MIN = mybir.AluOpType.min
    MULT = mybir.AluOpType.mult
    ADD = mybir.AluOpType.add
    SQ = mybir.ActivationFunctionType.Square

    nc = bacc.Bacc(
        "TRN2",
        target_bir_lowering=False,
        debug=False,
        enable_asserts=False,
        num_devices=N_CORES,
    )

    # packed input: cols 0:96 = grid (y, (c,x)), cols 96:190 = rmat^T [32, 94]
    gr = nc.dram_tensor("gr", (H, C * W + OH), f16, kind="ExternalInput").ap()
    pcs = nc.dram_tensor("a_aug", (5, M_CORE), f16, kind="ExternalInput").ap()
    out_min = nc.dram_tensor("minaug", (128, PC_TILES), f32, kind="ExternalOutput").ap()

    with tile.TileContext(nc) as tc:
        with tc.tile_pool(name="const", bufs=1) as cpool:

            # ---------------- load inputs (parallel queues) ----------------
            gr_sb = cpool.tile([H, C * W + OH], f16)     # [32, 190]
            nc.sync.dma_start(out=gr_sb[:], in_=gr)
            g_sb = gr_sb[:, 0:C * W]
            rm_sb = gr_sb[:, C * W:]
            aaug = cpool.tile([5, M_CORE], f16)          # host-built lhsT rows
            nc.scalar.dma_start(out=aaug[:], in_=pcs)

            # ---------------- mesh refine on PE (fp32, exact) ----------------
            bstage = cpool.tile([OH, 5 * OH], f16)       # [94, (5c, 94w)] rhs rows
            sq0 = cpool.tile([OH, OH], f32)
            sq1 = cpool.tile([OH, OH], f32)
            sq2 = cpool.tile([OH, OH], f32)
            sq01 = cpool.tile([OH, OH], f32)
            ssum = cpool.tile([OH, OH], f32)
            a_sb = cpool.tile([H, C * OH], f16)          # [32x, (c,oh)]

            with tc.tile_pool(name="rpsum", bufs=4, space="PSUM") as rpool:
                for c in range(C):
                    pA = rpool.tile([H, OH], f32, name="pA")   # [x, oh]
                    nc.tensor.matmul(
                        out=pA[:],
                        lhsT=g_sb[:, c * W:(c + 1) * W],       # [y, x] ch c
                        rhs=rm_sb,
                        start=True, stop=True)
                    nc.vector.tensor_copy(a_sb[:, c * OH:(c + 1) * OH], pA[:])
                sqs = [sq0, sq1, sq2]
                for c in range(C):
                    pB = rpool.tile([OH, OH], f32, name="pB")  # [oh, ow] ch c
                    nc.tensor.matmul(
                        out=pB[:],
                        lhsT=a_sb[:, c * OH:(c + 1) * OH],
                        rhs=rm_sb,
                        start=True, stop=True,
                    )
                    # m~ = fp16(m) into bstage block c (host supplies -2p~
                    # on the pc side); square the fp16 values on DVE so
                    # s = ||m~||^2 is exact in fp32
                    mblk = bstage[:, c * OH:(c + 1) * OH]
                    nc.scalar.copy(mblk, pB[:])
                    nc.vector.tensor_tensor(out=sqs[c][:], in0=mblk, in1=mblk,
                                            op=MULT)
                nc.vector.tensor_tensor(out=sq01[:], in0=sq0[:], in1=sq1[:], op=ADD)
                nc.vector.tensor_tensor(out=ssum[:], in0=sq01[:], in1=sq2[:], op=ADD)
                # s_hi = fp16(s); s_lo = fp16(s - s_hi)
                sh = bstage[:, 3 * OH:4 * OH]
                sl = bstage[:, 4 * OH:5 * OH]
                nc.scalar.copy(sh, ssum[:])
                nc.vector.scalar_tensor_tensor(
                    out=sl, in0=ssum[:], scalar=1.0, in1=sh, op0=MULT,
                    op1=mybir.AluOpType.subtract)

                if n_warm:
                    warm = rpool.tile([OH, OH], f32, name="warm")
                    for _ in range(n_warm):
                        nc.tensor.matmul(out=warm[:], lhsT=a_sb[:, 0:OH],
                                         rhs=rm_sb, start=True, stop=True)

            # ------------- flatten SBUF->SBUF in row slabs -------------
            # baug[c, h*94+w] = bstage[h, (c, w)]; one DMA per (c, slab):
            # src [hh partitions, 94 cols] pairs positionally with dst
            # [1 partition, hh*94] (both h-major).
            baug = cpool.tile([5, N_MESH], f16)
            qs = [nc.gpsimd, nc.sync, nc.scalar, nc.gpsimd, nc.sync]
            for (h0, h1) in SLABS:
                for c in range(5):
                    qs[c].dma_start(
                        out=baug[c:c + 1, h0 * OH:h1 * OH],
                        in_=bstage[h0:h1, c * OH:(c + 1) * OH],
                    )

            # ---------------- main loop ----------------
            # 1024-col PSUM windows from a 4-deep ring; per window either
            # D: DVE tensor_reduce(min) straight from PSUM, or
            # A: ACT copy PSUM->SBUF fp16 into a contiguous per-tile ebuf.
            # All of a tile's evac'd columns are then min-reduced by ONE
            # 4x-packed DVE tensor_scalar (accum min), plus a final small
            # reduce over the per-window partials.
            results = cpool.tile([128, PC_TILES], f32)
            trash16 = cpool.tile([128, 8836], f16)
            n_win = len(win_widths)
            gctr = 0
            with tc.tile_pool(name="mpsum", bufs=psum_bufs, space="PSUM") as mpool, \
                 tc.tile_pool(name="evac", bufs=3) as epool, \
                 tc.tile_pool(name="accp", bufs=4) as apool:
                for t in range(PC_TILES):
                    lh = aaug[:, t * 128:(t + 1) * 128]
                    ebuf = epool.tile([128, 8836], f16, name="ebuf")
                    accs = apool.tile([128, 10], f32, name="accs")
                    eoff = 0
                    nacc = 0
                    off = 0
                    for w, width in enumerate(win_widths):
                        path = pattern[gctr % len(pattern)]
                        gctr += 1
                        pd = mpool.tile([128, 1024], f32, name="pd")
                        fd = 0
                        while fd < width:
                            cw = min(512, width - fd)
                            nc.tensor.matmul(
                                out=pd[:, fd:fd + cw], lhsT=lh,
                                rhs=baug[:, off + fd:off + fd + cw],
                                start=True, stop=True)
                            fd += cw
                        if path == "D":
                            nc.vector.tensor_reduce(
                                accs[:, nacc:nacc + 1], pd[:, 0:width],
                                axis=mybir.AxisListType.X, op=MIN)
                            nacc += 1
                        else:  # A
                            nc.scalar.copy(ebuf[:, eoff:eoff + width],
                                           pd[:, 0:width])
                            eoff += width
                        off += width
                    if eoff:
                        nc.vector.tensor_scalar(
                            out=trash16[:, 0:eoff], in0=ebuf[:, 0:eoff],
                            scalar1=1e30, scalar2=None,
                            op0=MIN, op1=MIN,
                            accum_out=accs[:, nacc:nacc + 1])
                        nacc += 1
                    nc.vector.tensor_reduce(
                        results[:, t:t + 1], accs[:, 0:nacc],
                        axis=mybir.AxisListType.X, op=MIN)
                    if t == PC_TILES - 2:
                        nc.sync.dma_start(
                            out=out_min[:, 0:t + 1], in_=results[:, 0:t + 1])

            nc.gpsimd.dma_start(
                out=out_min[:, PC_TILES - 1:], in_=results[:, PC_TILES - 1:])

    nc.compile()
    return nc


def _get_nc():
    if "nc" not in _BUILT:
        _BUILT["nc"] = _build_kernel()
    return _BUILT["nc"]


def _make_a_aug(pc_slice: np.ndarray) -> np.ndarray:
    """Host-side marshalling of pc slice [3, M] fp32 into the fp16 augmented
    lhsT layout [5, M]: rows [-2*fp16(p) (3), 1, 1]."""
    m = pc_slice.shape[1]
    a = np.empty((5, m), dtype=np.float16)
    a[0:3] = -2.0 * pc_slice.astype(np.float16)
    a[3] = 1.0
    a[4] = 1.0
    return a


def _make_gr(grid_slice: np.ndarray, rmat_t: np.ndarray) -> np.ndarray:
    """Pack grid [3, 32, 32] (as [32, (c, x)]) and rmat^T [32, 94] into one
    fp16 [32, 190] input (the refine runs in fp16; its output is quantized
    to fp16 anyway, so this only adds same-magnitude perturbation)."""
    gr = np.empty((H, C * W + OH), dtype=np.float16)
    gr[:, :C * W] = grid_slice.transpose(1, 0, 2).reshape(H, C * W)
    gr[:, C * W:] = rmat_t
    return gr


def kernel(network_mesh: np.ndarray, pc: np.ndarray) -> np.ndarray:
    global LAST_RESULTS
    from concourse.bass_utils import run_bass_kernel_spmd

    network_mesh = np.ascontiguousarray(network_mesh, dtype=np.float32)
    pc = np.ascontiguousarray(pc, dtype=np.float32)

    nc = _get_nc()
    rmat_t = np.ascontiguousarray(_interp_matrix().T)   # [32, 94]

    in_maps = []
    for core in range(N_CORES):
        b, h = core // 2, core % 2
        in_maps.append({
            "gr": _make_gr(network_mesh[b], rmat_t),
            "a_aug": _make_a_aug(pc[b, :, h * M_CORE:(h + 1) * M_CORE]),
        })

    res = run_bass_kernel_spmd(nc, in_maps, core_ids=list(range(N_CORES)))
    LAST_RESULTS = res

    pcq = pc.astype(np.float16).astype(np.float32)       # p~ (device uses p~)
    pnorm = np.sum(pcq * pcq, axis=1)                    # ||p~||^2 fp32
    vals = []
    for core in range(N_CORES):
        b, h = core // 2, core % 2
        minaug = res.results[core]["minaug"]             # [128, 16]
        v = minaug.T.reshape(M_CORE)                     # point t*128+p order
        vals.append(v + pnorm[b, h * M_CORE:(h + 1) * M_CORE])
    dist2 = np.concatenate(vals)
    return np.array(np.mean(dist2, dtype=np.float32), dtype=np.float32)


# revision 6
# speedup vs baseline: 1.0118x; 1.0118x over previous
"""Trainium2 Bass kernel v2 for nn_MeshLoss2D (chamfer min-distance mesh loss).

Computation: refine a (B,3,32,32) mesh grid by bilinear factor 3 to (B,3,94,94),
then for every point-cloud point (B,3,4096) find min squared distance to any
refined mesh point, and return the mean over all B*4096 points.

Sharding: 8 cores = (batch b, pc half h); each core handles 2048 pc points of
one batch and that batch's full mesh (8836 points, exact - no padding).

v2 design (116129 ns baseline -> 105719 ns modeled):
  - 5-row fp16 "perturbed-exact" matmuls: quantize mesh and pc points to
    fp16 (m~, p~) and compute min_j d(p~, m~_j) EXACTLY: rows
    a=[-2p~(3), 1, 1], b=[m~(3), s_hi, s_lo] with s = ||m~||^2 computed in
    fp32 and split hi/lo fp16. fp16 x fp16 products accumulate exactly in
    PSUM fp32, so the only error is the ~1e-4 geometric perturbation
    (measured end-to-end rel err ~4e-4). Host adds back ||p~||^2. This
    replaces the old 12-row hi/lo split (fewer rows, simpler staging).
  - per pc-tile: 9 PSUM windows ([1024]*7 + [868, 800], exact 8836 cols, no
    padding) from a 4-deep [128,1024] PSUM ring, each routed through one of
    two paths by a tuned global pattern (8A:5D - ACT and DVE are the only
    engines that can read PSUM on real TRN2; GPSIMD tensor ops fail walrus
    codegen and DMA accum is add-only):
      D: DVE tensor_reduce(min) straight from PSUM       (~1.16 ns/col)
      A: ACT copy PSUM->SBUF fp16 into a contiguous      (~1.01 ns/col ACT)
         per-tile ebuf; ONE merged 4x-packed DVE
         tensor_scalar(min, accum min) per tile          (~0.27 ns/col DVE)
  - startup (was 21.7us, now ~12): single packed fp16 input DMA (grid+interp
    matrix); refine runs fp16 on the PE; staging reads refine PSUM directly
    (ACT copy to fp16 + DVE squares); SBUF->SBUF flatten as per-(row-slab,
    rhs-row) DMAs spread across SP/ACT/GPSIMD queues so the first windows
    start while later slabs are still in flight; split result writeback.
"""

import os
import sys

for _p in ("/opt/trn_rl_repo", "/opt/trn_rl_repo/concourse"):
    if _p not in sys.path:
        sys.path.insert(0, _p)

import numpy as np

B, C, H, W = 4, 3, 32, 32
FACTOR = 3
OH = (H - 1) * FACTOR + 1        # 94
N_MESH = OH * OH                 # 8836
M_TOTAL = 4096
N_CORES = 8
M_CORE = M_TOTAL * B // N_CORES  # 2048 pc points per core
PC_TILES = M_CORE // 128         # 16

# window widths per pc-tile: 7*1024 + 868 + 800 = 8836; every matmul chunk
# starts on a PSUM bank boundary (matmuls may not cross banks)
WIN_WIDTHS = [1024] * 7 + [868, 800]
# global rotating window pattern, tuned 8A:5D (see module docstring)
PATTERN = "DAADADAADADAA"
# flatten row slabs (rows of the 94x94 refined grid per DMA)
SLABS = [(0, 12), (12, 40), (40, 94)]
N_WARM = 0                       # dummy PE matmuls (tested: do not help)

_BUILT = {}
LAST_RESULTS = None


def _interp_matrix():
    """R [OH, H] fp32 with R[o, y0]=1-w, R[o, y0+1]=w replicating reference
    fp32 arithmetic (ys = arange(oh)/3 in fp32)."""
    ys = np.arange(OH, dtype=np.float32) / np.float32(FACTOR)
    y0 = np.clip(np.floor(ys).astype(np.int64), 0, H - 2)
    wy = ys - y0.astype(np.float32)
    R = np.zeros((OH, H), dtype=np.float32)
    R[np.arange(OH), y0] = np.float32(1.0) - wy
    R[np.arange(OH), y0 + 1] += wy
    return R


def _chunks(width):
    """Matmul chunk widths: each starts on a 512-col PSUM bank boundary and
    is >= 256 wide so f32r runs at full rate."""
    assert 768 < width <= 1024
    return [512, width - 512]


def _build_kernel(pattern=PATTERN, win_widths=None, psum_bufs=4, evac_bufs=6,
                  n_warm=N_WARM):
    from concourse import bacc, mybir
    import concourse.tile as tile

    if win_widths is None:
        win_widths = WIN_WIDTHS
    assert sum(win_widths) == N_MESH

    f32 = mybir.dt.float32
    f32r = mybir.dt.float32r
    f16 = mybir.dt.float16
    MIN = mybir.AluOpType.min
    MULT = mybir.AluOpType.mult
    ADD = mybir.AluOpType.add
    SQ = mybir.ActivationFunctionType.Square

    nc = bacc.Bacc(
        "TRN2",
        target_bir_lowering=False,
        debug=False,
        enable_asserts=False,
        num_devices=N_CORES,
    )

    # packed input: cols 0:96 = grid (y, (c,x)), cols 96:190 = rmat^T [32, 94]
    gr = nc.dram_tensor("gr", (H, C * W + OH), f16, kind="ExternalInput").ap()
    pcs = nc.dram_tensor("a_aug", (5, M_CORE), f16, kind="ExternalInput").ap()
    out_min = nc.dram_tensor("minaug", (128, PC_TILES), f32, kind="ExternalOutput").ap()

    with tile.TileContext(nc) as tc:
        with tc.tile_pool(name="const", bufs=1) as cpool:

            # ---------------- load inputs (parallel queues) ----------------
            gr_sb = cpool.tile([H, C * W + OH], f16)     # [32, 190]
            nc.sync.dma_start(out=gr_sb[:], in_=gr)
            g_sb = gr_sb[:, 0:C * W]
            rm_sb = gr_sb[:, C * W:]
            aaug = cpool.tile([5, M_CORE], f16)          # host-built lhsT rows
            nc.scalar.dma_start(out=aaug[:], in_=pcs)

            # ---------------- mesh refine on PE (fp32, exact) ----------------
            bstage = cpool.tile([OH, 5 * OH], f16)       # [94, (5c, 94w)] rhs rows
            sq0 = cpool.tile([OH, OH], f32)
            sq1 = cpool.tile([OH, OH], f32)
            sq2 = cpool.tile([OH, OH], f32)
            sq01 = cpool.tile([OH, OH], f32)
            ssum = cpool.tile([OH, OH], f32)
            a_sb = cpool.tile([H, C * OH], f16)          # [32x, (c,oh)]

            with tc.tile_pool(name="rpsum", bufs=4, space="PSUM") as rpool:
                for c in range(C):
                    pA = rpool.tile([H, OH], f32, name="pA")   # [x, oh]
                    nc.tensor.matmul(
                        out=pA[:],
                        lhsT=g_sb[:, c * W:(c + 1) * W],       # [y, x] ch c
                        rhs=rm_sb,
                        start=True, stop=True)
                    nc.vector.tensor_copy(a_sb[:, c * OH:(c + 1) * OH], pA[:])
                sqs = [sq0, sq1, sq2]
                for c in range(C):
                    pB = rpool.tile([OH, OH], f32, name="pB")  # [oh, ow] ch c
                    nc.tensor.matmul(
                        out=pB[:],
                        lhsT=a_sb[:, c * OH:(c + 1) * OH],
                        rhs=rm_sb,
                        start=True, stop=True,
                    )
                    # m~ = fp16(m) into bstage block c (host supplies -2p~
                    # on the pc side); square the fp16 values on DVE so
                    # s = ||m~||^2 is exact in fp32
                    mblk = bstage[:, c * OH:(c + 1) * OH]
                    nc.scalar.copy(mblk, pB[:])
                    nc.vector.tensor_tensor(out=sqs[c][:], in0=mblk, in1=mblk,
                                            op=MULT)
                nc.vector.tensor_tensor(out=sq01[:], in0=sq0[:], in1=sq1[:], op=ADD)
                nc.vector.tensor_tensor(out=ssum[:], in0=sq01[:], in1=sq2[:], op=ADD)
                # s_hi = fp16(s); s_lo = fp16(s - s_hi)
                sh = bstage[:, 3 * OH:4 * OH]
                sl = bstage[:, 4 * OH:5 * OH]
                nc.scalar.copy(sh, ssum[:])
                nc.vector.scalar_tensor_tensor(
                    out=sl, in0=ssum[:], scalar=1.0, in1=sh, op0=MULT,
                    op1=mybir.AluOpType.subtract)

                if n_warm:
                    warm = rpool.tile([OH, OH], f32, name="warm")
                    for _ in range(n_warm):
                        nc.tensor.matmul(out=warm[:], lhsT=a_sb[:, 0:OH],
                                         rhs=rm_sb, start=True, stop=True)

            # ------------- flatten SBUF->SBUF in row slabs -------------
            # baug[c, h*94+w] = bstage[h, (c, w)]; one DMA per (c, slab):
            # src [hh partitions, 94 cols] pairs positionally with dst
            # [1 partition, hh*94] (both h-major).
            baug = cpool.tile([5, N_MESH], f16)
            qs = [nc.gpsimd, nc.sync, nc.scalar, nc.gpsimd, nc.sync]
            for (h0, h1) in SLABS:
                for c in range(5):
                    qs[c].dma_start(
                        out=baug[c:c + 1, h0 * OH:h1 * OH],
                        in_=bstage[h0:h1, c * OH:(c + 1) * OH],
                    )

            # ---------------- main loop ----------------
            # 1024-col PSUM windows from a 4-deep ring; per window either
            # D: DVE tensor_reduce(min) straight from PSUM, or
            # A: ACT copy PSUM->SBUF fp16 into a contiguous per-tile ebuf.
            # All of a tile's evac'd columns are then min-reduced by ONE
            # 4x-packed DVE tensor_scalar (accum min), plus a final small
            # reduce over the per-window partials.
            results = cpool.tile([128, PC_TILES], f32)
            trash16 = cpool.tile([128, 8836], f16)
            n_win = len(win_widths)
            gctr = 0
            with tc.tile_pool(name="mpsum", bufs=psum_bufs, space="PSUM") as mpool, \
                 tc.tile_pool(name="evac", bufs=3) as epool, \
                 tc.tile_pool(name="accp", bufs=4) as apool:
                for t in range(PC_TILES):
                    lh = aaug[:, t * 128:(t + 1) * 128]
                    ebuf = epool.tile([128, 8836], f16, name="ebuf")
                    accs = apool.tile([128, 10], f32, name="accs")
                    eoff = 0
                    nacc = 0
                    off = 0
                    for w, width in enumerate(win_widths):
                        path = pattern[gctr % len(pattern)]
                        if t == 0:
                            # ramp tile: interleave A early so ACT (the
                            # binding engine) starts as soon as possible
                            path = "ADADADADD"[w]
                        gctr += 1
                        pd = mpool.tile([128, 1024], f32, name="pd")
                        fd = 0
                        while fd < width:
                            cw = min(512, width - fd)
                            nc.tensor.matmul(
                                out=pd[:, fd:fd + cw], lhsT=lh,
                                rhs=baug[:, off + fd:off + fd + cw],
                                start=True, stop=True)
                            fd += cw
                        if path == "D":
                            nc.vector.tensor_reduce(
                                accs[:, nacc:nacc + 1], pd[:, 0:width],
                                axis=mybir.AxisListType.X, op=MIN)
                            nacc += 1
                        else:  # A
                            nc.scalar.copy(ebuf[:, eoff:eoff + width],
                                           pd[:, 0:width])
                            eoff += width
                        off += width
                    if eoff:
                        nc.vector.tensor_scalar(
                            out=trash16[:, 0:eoff], in0=ebuf[:, 0:eoff],
                            scalar1=1e30, scalar2=None,
                            op0=MIN, op1=MIN,
                            accum_out=accs[:, nacc:nacc + 1])
                        nacc += 1
                    nc.vector.tensor_reduce(
                        results[:, t:t + 1], accs[:, 0:nacc],
                        axis=mybir.AxisListType.X, op=MIN)
                    if t == PC_TILES - 2:
                        nc.sync.dma_start(
                            out=out_min[:, 0:t + 1], in_=results[:, 0:t + 1])

            nc.sync.dma_start(
                out=out_min[:, PC_TILES - 1:], in_=results[:, PC_TILES - 1:])

    nc.compile()
    return nc


def _get_nc():
    if "nc" not in _BUILT:
        _BUILT["nc"] = _build_kernel()
    return _BUILT["nc"]


def _make_a_aug(pc_slice: np.ndarray) -> np.ndarray:
    """Host-side marshalling of pc slice [3, M] fp32 into the fp16 augmented
    lhsT layout [5, M]: rows [-2*fp16(p) (3), 1, 1]."""
    m = pc_slice.shape[1]
    a = np.empty((5, m), dtype=np.float16)
    a[0:3] = -2.0 * pc_slice.astype(np.float16)
    a[3] = 1.0
    a[4] = 1.0
    return a


def _make_gr(grid_slice: np.ndarray, rmat_t: np.ndarray) -> np.ndarray:
    """Pack grid [3, 32, 32] (as [32, (c, x)]) and rmat^T [32, 94] into one
    fp16 [32, 190] input (the refine runs in fp16; its output is quantized
    to fp16 anyway, so this only adds same-magnitude perturbation)."""
    gr = np.empty((H, C * W + OH), dtype=np.float16)
    gr[:, :C * W] = grid_slice.transpose(1, 0, 2).reshape(H, C * W)
    gr[:, C * W:] = rmat_t
    return gr


def kernel(network_mesh: np.ndarray, pc: np.ndarray) -> np.ndarray:
    global LAST_RESULTS
    from concourse.bass_utils import run_bass_kernel_spmd

    network_mesh = np.ascontiguousarray(network_mesh, dtype=np.float32)
    pc = np.ascontiguousarray(pc, dtype=np.float32)

    nc = _get_nc()
    rmat_t = np.ascontiguousarray(_interp_matrix().T)   # [32, 94]

    in_maps = []
    for core in range(N_CORES):
        b, h = core // 2, core % 2
        in_maps.append({
            "gr": _make_gr(network_mesh[b], rmat_t),
            "a_aug": _make_a_aug(pc[b, :, h * M_CORE:(h + 1) * M_CORE]),
        })

    res = run_bass_kernel_spmd(nc, in_maps, core_ids=list(range(N_CORES)))
    LAST_RESULTS = res

    pcq = pc.astype(np.float16).astype(np.float32)       # p~ (device uses p~)
    pnorm = np.sum(pcq * pcq, axis=1)                    # ||p~||^2 fp32
    vals = []
    for core in range(N_CORES):
        b, h = core // 2, core % 2
        minaug = res.results[core]["minaug"]             # [128, 16]
        v = minaug.T.reshape(M_CORE)                     # point t*128+p order
        vals.append(v + pnorm[b, h * M_CORE:(h + 1) * M_CORE])
    dist2 = np.concatenate(vals)
    return np.array(np.mean(dist2, dtype=np.float32), dtype=np.float32)


# revision 8
# speedup vs baseline: 1.0158x; 1.0040x over previous
"""Trainium2 Bass kernel v2 for nn_MeshLoss2D (chamfer min-distance mesh loss).

Computation: refine a (B,3,32,32) mesh grid by bilinear factor 3 to (B,3,94,94),
then for every point-cloud point (B,3,4096) find min squared distance to any
refined mesh point, and return the mean over all B*4096 points.

Sharding: 8 cores = (batch b, pc half h); each core handles 2048 pc points of
one batch and that batch's full mesh (8836 points, exact - no padding).

v2 design (116129 ns baseline -> 104071 ns modeled):
  - 5-row fp16 "perturbed-exact" matmuls: quantize mesh and pc points to
    fp16 (m~, p~) and compute min_j d(p~, m~_j) EXACTLY: rows
    a=[-2p~(3), 1, 1], b=[m~(3), s_hi, s_lo] with s = ||m~||^2 computed in
    fp32 and split hi/lo fp16. fp16 x fp16 products accumulate exactly in
    PSUM fp32, so the only error is the ~1e-4 geometric perturbation
    (measured end-to-end rel err ~4e-4). Host adds back ||p~||^2. This
    replaces the old 12-row hi/lo split (fewer rows, simpler staging).
  - per pc-tile: 9 PSUM windows ([1024]*7 + [868, 800], exact 8836 cols, no
    padding) from a 4-deep [128,1024] PSUM ring, each routed through one of
    two paths by a tuned global pattern (8A:5D - ACT and DVE are the only
    engines that can read PSUM on real TRN2; GPSIMD tensor ops fail walrus
    codegen and DMA accum is add-only):
      D: DVE tensor_reduce(min) straight from PSUM       (~1.16 ns/col)
    Tile 0 uses an A-interleaved ramp pattern so ACT starts immediately;
    the final result DMA rides the idle SP queue to shorten the tail.
      A: ACT copy PSUM->SBUF fp16 into a contiguous      (~1.01 ns/col ACT)
         per-tile ebuf; ONE merged 4x-packed DVE
         tensor_scalar(min, accum min) per tile          (~0.27 ns/col DVE)
    D results land as single fp16 columns inside ebuf, so the one merged
    reduce per tile also folds them in and writes results directly (no
    per-tile final reduce).
  - startup (was 21.7us, now ~12): single packed fp16 input DMA (grid+interp
    matrix); refine runs fp16 on the PE; staging reads refine PSUM directly
    (ACT copy to fp16 + DVE squares); SBUF->SBUF flatten as per-(row-slab,
    rhs-row) DMAs spread across SP/ACT/GPSIMD queues so the first windows
    start while later slabs are still in flight; split result writeback.
"""

import os
import sys

for _p in ("/opt/trn_rl_repo", "/opt/trn_rl_repo/concourse"):
    if _p not in sys.path:
        sys.path.insert(0, _p)

import numpy as np

B, C, H, W = 4, 3, 32, 32
FACTOR = 3
OH = (H - 1) * FACTOR + 1        # 94
N_MESH = OH * OH                 # 8836
M_TOTAL = 4096
N_CORES = 8
M_CORE = M_TOTAL * B // N_CORES  # 2048 pc points per core
PC_TILES = M_CORE // 128         # 16

# window widths per pc-tile: 7*1024 + 868 + 800 = 8836; every matmul chunk
# starts on a PSUM bank boundary (matmuls may not cross banks)
WIN_WIDTHS = [1024] * 7 + [868, 800]
# global rotating window pattern, tuned 8A:5D (see module docstring)
PATTERN = "DAADADAADADAA"
# flatten row slabs (rows of the 94x94 refined grid per DMA)
SLABS = [(0, 12), (12, 40), (40, 94)]
N_WARM = 0                       # dummy PE matmuls (tested: do not help)

_BUILT = {}
LAST_RESULTS = None


def _interp_matrix():
    """R [OH, H] fp32 with R[o, y0]=1-w, R[o, y0+1]=w replicating reference
    fp32 arithmetic (ys = arange(oh)/3 in fp32)."""
    ys = np.arange(OH, dtype=np.float32) / np.float32(FACTOR)
    y0 = np.clip(np.floor(ys).astype(np.int64), 0, H - 2)
    wy = ys - y0.astype(np.float32)
    R = np.zeros((OH, H), dtype=np.float32)
    R[np.arange(OH), y0] = np.float32(1.0) - wy
    R[np.arange(OH), y0 + 1] += wy
    return R


def _chunks(width):
    """Matmul chunk widths: each starts on a 512-col PSUM bank boundary and
    is >= 256 wide so f32r runs at full rate."""
    assert 768 < width <= 1024
    return [512, width - 512]


def _build_kernel(pattern=PATTERN, win_widths=None, psum_bufs=4, evac_bufs=6,
                  n_warm=N_WARM):
    from concourse import bacc, mybir
    import concourse.tile as tile

    if win_widths is None:
        win_widths = WIN_WIDTHS
    assert sum(win_widths) == N_MESH

    f32 = mybir.dt.float32
    f32r = mybir.dt.float32r
    f16 = mybir.dt.float16
    MIN = mybir.AluOpType.min
    MULT = mybir.AluOpType.mult
    ADD = mybir.AluOpType.add
    SQ = mybir.ActivationFunctionType.Square

    nc = bacc.Bacc(
        "TRN2",
        target_bir_lowering=False,
        debug=False,
        enable_asserts=False,
        num_devices=N_CORES,
    )

    # packed input: cols 0:96 = grid (y, (c,x)), cols 96:190 = rmat^T [32, 94]
    gr = nc.dram_tensor("gr", (H, C * W + OH), f16, kind="ExternalInput").ap()
    pcs = nc.dram_tensor("a_aug", (5, M_CORE), f16, kind="ExternalInput").ap()
    out_min = nc.dram_tensor("minaug", (128, PC_TILES), f32, kind="ExternalOutput").ap()

    with tile.TileContext(nc) as tc:
        with tc.tile_pool(name="const", bufs=1) as cpool:

            # ---------------- load inputs (parallel queues) ----------------
            gr_sb = cpool.tile([H, C * W + OH], f16)     # [32, 190]
            nc.sync.dma_start(out=gr_sb[:], in_=gr)
            g_sb = gr_sb[:, 0:C * W]
            rm_sb = gr_sb[:, C * W:]
            aaug = cpool.tile([5, M_CORE], f16)          # host-built lhsT rows
            nc.scalar.dma_start(out=aaug[:], in_=pcs)

            # ---------------- mesh refine on PE (fp32, exact) ----------------
            bstage = cpool.tile([OH, 5 * OH], f16)       # [94, (5c, 94w)] rhs rows
            sq0 = cpool.tile([OH, OH], f32)
            sq1 = cpool.tile([OH, OH], f32)
            sq2 = cpool.tile([OH, OH], f32)
            sq01 = cpool.tile([OH, OH], f32)
            ssum = cpool.tile([OH, OH], f32)
            a_sb = cpool.tile([H, C * OH], f16)          # [32x, (c,oh)]

            with tc.tile_pool(name="rpsum", bufs=4, space="PSUM") as rpool:
                for c in range(C):
                    pA = rpool.tile([H, OH], f32, name="pA")   # [x, oh]
                    nc.tensor.matmul(
                        out=pA[:],
                        lhsT=g_sb[:, c * W:(c + 1) * W],       # [y, x] ch c
                        rhs=rm_sb,
                        start=True, stop=True)
                    nc.vector.tensor_copy(a_sb[:, c * OH:(c + 1) * OH], pA[:])
                sqs = [sq0, sq1, sq2]
                for c in range(C):
                    pB = rpool.tile([OH, OH], f32, name="pB")  # [oh, ow] ch c
                    nc.tensor.matmul(
                        out=pB[:],
                        lhsT=a_sb[:, c * OH:(c + 1) * OH],
                        rhs=rm_sb,
                        start=True, stop=True,
                    )
                    # m~ = fp16(m) into bstage block c (host supplies -2p~
                    # on the pc side); square the fp16 values on DVE so
                    # s = ||m~||^2 is exact in fp32
                    mblk = bstage[:, c * OH:(c + 1) * OH]
                    nc.scalar.copy(mblk, pB[:])
                    nc.vector.tensor_tensor(out=sqs[c][:], in0=mblk, in1=mblk,
                                            op=MULT)
                nc.vector.tensor_tensor(out=sq01[:], in0=sq0[:], in1=sq1[:], op=ADD)
                nc.vector.tensor_tensor(out=ssum[:], in0=sq01[:], in1=sq2[:], op=ADD)
                # s_hi = fp16(s); s_lo = fp16(s - s_hi)
                sh = bstage[:, 3 * OH:4 * OH]
                sl = bstage[:, 4 * OH:5 * OH]
                nc.scalar.copy(sh, ssum[:])
                nc.vector.scalar_tensor_tensor(
                    out=sl, in0=ssum[:], scalar=1.0, in1=sh, op0=MULT,
                    op1=mybir.AluOpType.subtract)

                if n_warm:
                    warm = rpool.tile([OH, OH], f32, name="warm")
                    for _ in range(n_warm):
                        nc.tensor.matmul(out=warm[:], lhsT=a_sb[:, 0:OH],
                                         rhs=rm_sb, start=True, stop=True)

            # ------------- flatten SBUF->SBUF in row slabs -------------
            # baug[c, h*94+w] = bstage[h, (c, w)]; one DMA per (c, slab):
            # src [hh partitions, 94 cols] pairs positionally with dst
            # [1 partition, hh*94] (both h-major).
            baug = cpool.tile([5, N_MESH], f16)
            qs = [nc.gpsimd, nc.sync, nc.scalar, nc.gpsimd, nc.sync]
            for (h0, h1) in SLABS:
                for c in range(5):
                    qs[c].dma_start(
                        out=baug[c:c + 1, h0 * OH:h1 * OH],
                        in_=bstage[h0:h1, c * OH:(c + 1) * OH],
                    )

            # ---------------- main loop ----------------
            # 1024-col PSUM windows from a 4-deep ring; per window either
            # D: DVE tensor_reduce(min) straight from PSUM, or
            # A: ACT copy PSUM->SBUF fp16 into a contiguous per-tile ebuf.
            # All of a tile's evac'd columns are then min-reduced by ONE
            # 4x-packed DVE tensor_scalar (accum min), plus a final small
            # reduce over the per-window partials.
            results = cpool.tile([128, PC_TILES], f32)
            trash16 = cpool.tile([128, 8836], f16)
            n_win = len(win_widths)
            gctr = 0
            with tc.tile_pool(name="mpsum", bufs=psum_bufs, space="PSUM") as mpool, \
                 tc.tile_pool(name="evac", bufs=3) as epool, \
                 tc.tile_pool(name="accp", bufs=4) as apool:
                for t in range(PC_TILES):
                    lh = aaug[:, t * 128:(t + 1) * 128]
                    ebuf = epool.tile([128, 8848], f16, name="ebuf")
                    eoff = 0
                    off = 0
                    for w, width in enumerate(win_widths):
                        path = pattern[gctr % len(pattern)]
                        if t == 0:
                            # ramp tile: interleave A early so ACT (the
                            # binding engine) starts as soon as possible
                            path = "ADADADADD"[w]
                        gctr += 1
                        pd = mpool.tile([128, 1024], f32, name="pd")
                        fd = 0
                        while fd < width:
                            cw = min(512, width - fd)
                            nc.tensor.matmul(
                                out=pd[:, fd:fd + cw], lhsT=lh,
                                rhs=baug[:, off + fd:off + fd + cw],
                                start=True, stop=True)
                            fd += cw
                        if path == "D":
                            # direct min lands as ONE fp16 col in ebuf so the
                            # merged reduce below folds it in for free
                            nc.vector.tensor_reduce(
                                ebuf[:, eoff:eoff + 1], pd[:, 0:width],
                                axis=mybir.AxisListType.X, op=MIN)
                            eoff += 1
                        else:  # A
                            nc.scalar.copy(ebuf[:, eoff:eoff + width],
                                           pd[:, 0:width])
                            eoff += width
                        off += width
                    nc.vector.tensor_scalar(
                        out=trash16[:, 0:eoff], in0=ebuf[:, 0:eoff],
                        scalar1=1e30, scalar2=None,
                        op0=MIN, op1=MIN,
                        accum_out=results[:, t:t + 1])
                    if t == PC_TILES - 2:
                        nc.sync.dma_start(
                            out=out_min[:, 0:t + 1], in_=results[:, 0:t + 1])

            nc.sync.dma_start(
                out=out_min[:, PC_TILES - 1:], in_=results[:, PC_TILES - 1:])

    nc.compile()
    return nc


def _get_nc():
    if "nc" not in _BUILT:
        _BUILT["nc"] = _build_kernel()
    return _BUILT["nc"]


def _make_a_aug(pc_slice: np.ndarray) -> np.ndarray:
    """Host-side marshalling of pc slice [3, M] fp32 into the fp16 augmented
    lhsT layout [5, M]: rows [-2*fp16(p) (3), 1, 1]."""
    m = pc_slice.shape[1]
    a = np.empty((5, m), dtype=np.float16)
    a[0:3] = -2.0 * pc_slice.astype(np.float16)
    a[3] = 1.0
    a[4] = 1.0
    return a


def _make_gr(grid_slice: np.ndarray, rmat_t: np.ndarray) -> np.ndarray:
    """Pack grid [3, 32, 32] (as [32, (c, x)]) and rmat^T [32, 94] into one
    fp16 [32, 190] input (the refine runs in fp16; its output is quantized
    to fp16 anyway, so this only adds same-magnitude perturbation)."""
    gr = np.empty((H, C * W + OH), dtype=np.float16)
    gr[:, :C * W] = grid_slice.transpose(1, 0, 2).reshape(H, C * W)
    gr[:, C * W:] = rmat_t
    return gr


def kernel(network_mesh: np.ndarray, pc: np.ndarray) -> np.ndarray:
    global LAST_RESULTS
    from concourse.bass_utils import run_bass_kernel_spmd

    network_mesh = np.ascontiguousarray(network_mesh, dtype=np.float32)
    pc = np.ascontiguousarray(pc, dtype=np.float32)

    nc = _get_nc()
    rmat_t = np.ascontiguousarray(_interp_matrix().T)   # [32, 94]

    in_maps = []
    for core in range(N_CORES):
        b, h = core // 2, core % 2
        in_maps.append({
            "gr": _make_gr(network_mesh[b], rmat_t),
            "a_aug": _make_a_aug(pc[b, :, h * M_CORE:(h + 1) * M_CORE]),
        })

    res = run_bass_kernel_spmd(nc, in_maps, core_ids=list(range(N_CORES)))
    LAST_RESULTS = res

    pcq = pc.astype(np.float16).astype(np.float32)       # p~ (device uses p~)
    pnorm = np.sum(pcq * pcq, axis=1)                    # ||p~||^2 fp32
    vals = []
    for core in range(N_CORES):
        b, h = core // 2, core % 2
        minaug = res.results[core]["minaug"]             # [128, 16]
        v = minaug.T.reshape(M_CORE)                     # point t*128+p order
        vals.append(v + pnorm[b, h * M_CORE:(h + 1) * M_CORE])
    dist2 = np.concatenate(vals)
    return np.array(np.mean(dist2, dtype=np.float32), dtype=np.float32)


# revision 9
# speedup vs baseline: 1.0195x; 1.0036x over previous
"""Trainium2 Bass kernel v2 for nn_MeshLoss2D (chamfer min-distance mesh loss).

Computation: refine a (B,3,32,32) mesh grid by bilinear factor 3 to (B,3,94,94),
then for every point-cloud point (B,3,4096) find min squared distance to any
refined mesh point, and return the mean over all B*4096 points.

Sharding: 8 cores = (batch b, pc half h); each core handles 2048 pc points of
one batch and that batch's full mesh (8836 points, exact - no padding).

v2 design (116129 ns baseline -> 103693 ns modeled):
  - 5-row fp16 "perturbed-exact" matmuls: quantize mesh and pc points to
    fp16 (m~, p~) and compute min_j d(p~, m~_j) EXACTLY: rows
    a=[-2p~(3), 1, 1], b=[m~(3), s_hi, s_lo] with s = ||m~||^2 computed in
    fp32 and split hi/lo fp16. fp16 x fp16 products accumulate exactly in
    PSUM fp32, so the only error is the ~1e-4 geometric perturbation
    (measured end-to-end rel err ~4e-4). Host adds back ||p~||^2. This
    replaces the old 12-row hi/lo split (fewer rows, simpler staging).
  - per pc-tile: 9 PSUM windows ([1024]*7 + [868, 800], exact 8836 cols, no
    padding) from a 4-deep [128,1024] PSUM ring, each routed through one of
    two paths by a tuned global pattern (8A:5D - ACT and DVE are the only
    engines that can read PSUM on real TRN2; GPSIMD tensor ops fail walrus
    codegen and DMA accum is add-only):
      D: DVE tensor_reduce(min) straight from PSUM       (~1.16 ns/col)
    Tile 0 uses an A-interleaved ramp pattern so ACT starts immediately;
    the final result DMA rides the idle SP queue to shorten the tail.
      A: ACT copy PSUM->SBUF fp16 into a contiguous      (~1.01 ns/col ACT)
         per-tile ebuf; ONE merged 4x-packed DVE
         tensor_scalar(min, accum min) per tile          (~0.27 ns/col DVE)
    D results land as single fp16 columns inside ebuf, so the one merged
    reduce per tile also folds them in and writes results directly (no
    per-tile final reduce).
  - startup (was 21.7us, now ~12): single packed fp16 input DMA (grid+interp
    matrix); refine runs fp16 on the PE; staging reads refine PSUM directly
    (ACT copy to fp16 + DVE squares); SBUF->SBUF flatten as per-(row-slab,
    rhs-row) DMAs spread across SP/ACT/GPSIMD queues so the first windows
    start while later slabs are still in flight; split result writeback.
"""

import os
import sys

for _p in ("/opt/trn_rl_repo", "/opt/trn_rl_repo/concourse"):
    if _p not in sys.path:
        sys.path.insert(0, _p)

import numpy as np

B, C, H, W = 4, 3, 32, 32
FACTOR = 3
OH = (H - 1) * FACTOR + 1        # 94
N_MESH = OH * OH                 # 8836
M_TOTAL = 4096
N_CORES = 8
M_CORE = M_TOTAL * B // N_CORES  # 2048 pc points per core
PC_TILES = M_CORE // 128         # 16

# window widths per pc-tile: 7*1024 + 868 + 800 = 8836; every matmul chunk
# starts on a PSUM bank boundary (matmuls may not cross banks)
WIN_WIDTHS = [1024] * 7 + [868, 800]
# global rotating window pattern, tuned 8A:5D (see module docstring)
PATTERN = "DAADADAADADAA"
# flatten row slabs (rows of the 94x94 refined grid per DMA)
SLABS = [(0, 12), (12, 40), (40, 94)]
N_WARM = 0                       # dummy PE matmuls (tested: do not help)

_BUILT = {}
LAST_RESULTS = None


def _interp_matrix():
    """R [OH, H] fp32 with R[o, y0]=1-w, R[o, y0+1]=w replicating reference
    fp32 arithmetic (ys = arange(oh)/3 in fp32)."""
    ys = np.arange(OH, dtype=np.float32) / np.float32(FACTOR)
    y0 = np.clip(np.floor(ys).astype(np.int64), 0, H - 2)
    wy = ys - y0.astype(np.float32)
    R = np.zeros((OH, H), dtype=np.float32)
    R[np.arange(OH), y0] = np.float32(1.0) - wy
    R[np.arange(OH), y0 + 1] += wy
    return R


def _chunks(width):
    """Matmul chunk widths: each starts on a 512-col PSUM bank boundary and
    is >= 256 wide so f32r runs at full rate."""
    assert 768 < width <= 1024
    return [512, width - 512]


def _build_kernel(pattern=PATTERN, win_widths=None, psum_bufs=4, evac_bufs=6,
                  n_warm=N_WARM):
    from concourse import bacc, mybir
    import concourse.tile as tile

    if win_widths is None:
        win_widths = WIN_WIDTHS
    assert sum(win_widths) == N_MESH

    f32 = mybir.dt.float32
    f32r = mybir.dt.float32r
    f16 = mybir.dt.float16
    MIN = mybir.AluOpType.min
    MULT = mybir.AluOpType.mult
    ADD = mybir.AluOpType.add
    SQ = mybir.ActivationFunctionType.Square

    nc = bacc.Bacc(
        "TRN2",
        target_bir_lowering=False,
        debug=False,
        enable_asserts=False,
        num_devices=N_CORES,
    )

    # packed input: cols 0:96 = grid (y, (c,x)), cols 96:190 = rmat^T [32, 94]
    gr = nc.dram_tensor("gr", (H, C * W + OH), f16, kind="ExternalInput").ap()
    pcs = nc.dram_tensor("a_aug", (5, M_CORE), f16, kind="ExternalInput").ap()
    out_min = nc.dram_tensor("minaug", (128, PC_TILES), f32, kind="ExternalOutput").ap()

    with tile.TileContext(nc) as tc:
        with tc.tile_pool(name="const", bufs=1) as cpool:

            # ---------------- load inputs (parallel queues) ----------------
            gr_sb = cpool.tile([H, C * W + OH], f16)     # [32, 190]
            nc.sync.dma_start(out=gr_sb[:], in_=gr)
            g_sb = gr_sb[:, 0:C * W]
            rm_sb = gr_sb[:, C * W:]
            aaug = cpool.tile([5, M_CORE], f16)          # host-built lhsT rows
            nc.scalar.dma_start(out=aaug[:], in_=pcs)

            # ---------------- mesh refine on PE (fp32, exact) ----------------
            bstage = cpool.tile([OH, 5 * OH], f16)       # [94, (5c, 94w)] rhs rows
            sq0 = cpool.tile([OH, OH], f32)
            sq1 = cpool.tile([OH, OH], f32)
            sq2 = cpool.tile([OH, OH], f32)
            sq01 = cpool.tile([OH, OH], f32)
            ssum = cpool.tile([OH, OH], f32)
            a_sb = cpool.tile([H, C * OH], f16)          # [32x, (c,oh)]

            with tc.tile_pool(name="rpsum", bufs=2, space="PSUM") as rpool:
                for c in range(C):
                    pA = rpool.tile([H, OH], f32, name="pA")   # [x, oh]
                    nc.tensor.matmul(
                        out=pA[:],
                        lhsT=g_sb[:, c * W:(c + 1) * W],       # [y, x] ch c
                        rhs=rm_sb,
                        start=True, stop=True)
                    nc.vector.tensor_copy(a_sb[:, c * OH:(c + 1) * OH], pA[:])
                sqs = [sq0, sq1, sq2]
                pBs = []
                for c in range(C):
                    pB = rpool.tile([OH, OH], f32, name=f"pB{c}")  # [oh, ow]
                    nc.tensor.matmul(
                        out=pB[:],
                        lhsT=a_sb[:, c * OH:(c + 1) * OH],
                        rhs=rm_sb,
                        start=True, stop=True,
                    )
                    pBs.append(pB)
                # m~ = fp16(m) into bstage blocks (host supplies -2p~ on the
                # pc side); square the fp16 values on DVE so s = ||m~||^2 is
                # exact in fp32. Staged in two row groups: rows [0:r0) first
                # so the first flatten slab (and with it the first main-loop
                # windows) can start while rows [r0:94) are still staging.
                # engine APs: base partition must be 32-aligned, and spans
                # from a non-zero base are capped at 32 partitions

                sh = bstage[:, 3 * OH:4 * OH]
                sl = bstage[:, 4 * OH:5 * OH]
                for lo, hi in ((0, 32), (32, 64), (64, OH)):
                    for c in range(C):
                        mblk = bstage[lo:hi, c * OH:(c + 1) * OH]
                        nc.scalar.copy(mblk, pBs[c][lo:hi, :])
                        nc.vector.tensor_tensor(out=sqs[c][lo:hi, :],
                                                in0=mblk, in1=mblk, op=MULT)
                    nc.vector.tensor_tensor(out=sq01[lo:hi, :],
                                            in0=sq0[lo:hi, :],
                                            in1=sq1[lo:hi, :], op=ADD)
                    nc.vector.tensor_tensor(out=ssum[lo:hi, :],
                                            in0=sq01[lo:hi, :],
                                            in1=sq2[lo:hi, :], op=ADD)
                    # s_hi = fp16(s); s_lo = fp16(s - s_hi)
                    nc.scalar.copy(sh[lo:hi, :], ssum[lo:hi, :])
                    nc.vector.scalar_tensor_tensor(
                        out=sl[lo:hi, :], in0=ssum[lo:hi, :], scalar=1.0,
                        in1=sh[lo:hi, :], op0=MULT,
                        op1=mybir.AluOpType.subtract)

                if n_warm:
                    warm = rpool.tile([OH, OH], f32, name="warm")
                    for _ in range(n_warm):
                        nc.tensor.matmul(out=warm[:], lhsT=a_sb[:, 0:OH],
                                         rhs=rm_sb, start=True, stop=True)

            # ------------- flatten SBUF->SBUF in row slabs -------------
            # baug[c, h*94+w] = bstage[h, (c, w)]; one DMA per (c, slab):
            # src [hh partitions, 94 cols] pairs positionally with dst
            # [1 partition, hh*94] (both h-major).
            baug = cpool.tile([5, N_MESH], f16)
            qs = [nc.gpsimd, nc.sync, nc.scalar, nc.gpsimd, nc.sync]
            for (h0, h1) in SLABS:
                for c in range(5):
                    qs[c].dma_start(
                        out=baug[c:c + 1, h0 * OH:h1 * OH],
                        in_=bstage[h0:h1, c * OH:(c + 1) * OH],
                    )

            # ---------------- main loop ----------------
            # 1024-col PSUM windows from a 4-deep ring; per window either
            # D: DVE tensor_reduce(min) straight from PSUM, or
            # A: ACT copy PSUM->SBUF fp16 into a contiguous per-tile ebuf.
            # All of a tile's evac'd columns are then min-reduced by ONE
            # 4x-packed DVE tensor_scalar (accum min), plus a final small
            # reduce over the per-window partials.
            results = cpool.tile([128, PC_TILES], f32)
            trash16 = cpool.tile([128, 8836], f16)
            n_win = len(win_widths)
            gctr = 0
            with tc.tile_pool(name="mpsum", bufs=psum_bufs, space="PSUM") as mpool, \
                 tc.tile_pool(name="evac", bufs=3) as epool, \
                 tc.tile_pool(name="accp", bufs=4) as apool:
                for t in range(PC_TILES):
                    lh = aaug[:, t * 128:(t + 1) * 128]
                    ebuf = epool.tile([128, 8848], f16, name="ebuf")
                    eoff = 0
                    off = 0
                    for w, width in enumerate(win_widths):
                        path = pattern[gctr % len(pattern)]
                        if t == 0:
                            # ramp tile: interleave A early so ACT (the
                            # binding engine) starts as soon as possible
                            path = "ADADADADD"[w]
                        gctr += 1
                        pd = mpool.tile([128, 1024], f32, name="pd")
                        fd = 0
                        while fd < width:
                            cw = min(512, width - fd)
                            nc.tensor.matmul(
                                out=pd[:, fd:fd + cw], lhsT=lh,
                                rhs=baug[:, off + fd:off + fd + cw],
                                start=True, stop=True)
                            fd += cw
                        if path == "D":
                            # direct min lands as ONE fp16 col in ebuf so the
                            # merged reduce below folds it in for free
                            nc.vector.tensor_reduce(
                                ebuf[:, eoff:eoff + 1], pd[:, 0:width],
                                axis=mybir.AxisListType.X, op=MIN)
                            eoff += 1
                        else:  # A
                            nc.scalar.copy(ebuf[:, eoff:eoff + width],
                                           pd[:, 0:width])
                            eoff += width
                        off += width
                    nc.vector.tensor_scalar(
                        out=trash16[:, 0:eoff], in0=ebuf[:, 0:eoff],
                        scalar1=1e30, scalar2=None,
                        op0=MIN, op1=MIN,
                        accum_out=results[:, t:t + 1])
                    if t == PC_TILES - 2:
                        nc.sync.dma_start(
                            out=out_min[:, 0:t + 1], in_=results[:, 0:t + 1])

            nc.sync.dma_start(
                out=out_min[:, PC_TILES - 1:], in_=results[:, PC_TILES - 1:])

    nc.compile()
    return nc


def _get_nc():
    if "nc" not in _BUILT:
        _BUILT["nc"] = _build_kernel()
    return _BUILT["nc"]


def _make_a_aug(pc_slice: np.ndarray) -> np.ndarray:
    """Host-side marshalling of pc slice [3, M] fp32 into the fp16 augmented
    lhsT layout [5, M]: rows [-2*fp16(p) (3), 1, 1]."""
    m = pc_slice.shape[1]
    a = np.empty((5, m), dtype=np.float16)
    a[0:3] = -2.0 * pc_slice.astype(np.float16)
    a[3] = 1.0
    a[4] = 1.0
    return a


def _make_gr(grid_slice: np.ndarray, rmat_t: np.ndarray) -> np.ndarray:
    """Pack grid [3, 32, 32] (as [32, (c, x)]) and rmat^T [32, 94] into one
    fp16 [32, 190] input (the refine runs in fp16; its output is quantized
    to fp16 anyway, so this only adds same-magnitude perturbation)."""
    gr = np.empty((H, C * W + OH), dtype=np.float16)
    gr[:, :C * W] = grid_slice.transpose(1, 0, 2).reshape(H, C * W)
    gr[:, C * W:] = rmat_t
    return gr


def kernel(network_mesh: np.ndarray, pc: np.ndarray) -> np.ndarray:
    global LAST_RESULTS
    from concourse.bass_utils import run_bass_kernel_spmd

    network_mesh = np.ascontiguousarray(network_mesh, dtype=np.float32)
    pc = np.ascontiguousarray(pc, dtype=np.float32)

    nc = _get_nc()
    rmat_t = np.ascontiguousarray(_interp_matrix().T)   # [32, 94]

    in_maps = []
    for core in range(N_CORES):
        b, h = core // 2, core % 2
        in_maps.append({
            "gr": _make_gr(network_mesh[b], rmat_t),
            "a_aug": _make_a_aug(pc[b, :, h * M_CORE:(h + 1) * M_CORE]),
        })

    res = run_bass_kernel_spmd(nc, in_maps, core_ids=list(range(N_CORES)))
    LAST_RESULTS = res

    pcq = pc.astype(np.float16).astype(np.float32)       # p~ (device uses p~)
    pnorm = np.sum(pcq * pcq, axis=1)                    # ||p~||^2 fp32
    vals = []
    for core in range(N_CORES):
        b, h = core // 2, core % 2
        minaug = res.results[core]["minaug"]             # [128, 16]
        v = minaug.T.reshape(M_CORE)                     # point t*128+p order
        vals.append(v + pnorm[b, h * M_CORE:(h + 1) * M_CORE])
    dist2 = np.concatenate(vals)
    return np.array(np.mean(dist2, dtype=np.float32), dtype=np.float32)
